# revision 64
# baseline (speedup 1.0000x reference)
"""Trainium2 Bass kernel: AttentionBlock (GroupNorm + cross-attention + residual).

Sharding: data-parallel over batch. b=8 maps 1:1 onto the 8 NeuronCores;
each core computes its whole batch item, no collectives.

Design (baseline 188.8us -> ~135.6us modeled):
  - fp8e4m3 DoubleRow matmuls (0.5 cycles/output-column, 2x contraction
    rows per instruction) for the Q/K/V/out projections and AV; dots run
    plain fp8 (per-head K=64 at partition offsets 0/64 - DR would need
    offset 96, which the ISA rejects). Channel pairings for every DR
    contraction are pre-interleaved in the host-side weight layouts.
  - x and ctx ship as bf16 from the host (half the DMA, exact enough for
    the residual); fp8 operands x8/q8/k8/v8/E8/avn8 feed the PE.
  - GroupNorm(x) is folded into the Q weights (wq8 = fp8(wqT*A), bqe =
    bq + wq@B). Group stats: blocks 0-1 of x via fused ACT passes
    (Copy/Square with accum_out - the Copy pass doubles as the x8 cast),
    blocks 2-3 via DVE bn_stats chasing split DMAs; gpsimd casts x8 for
    blocks 2-3 tile-by-tile ahead of the Q projection.
  - Softmax: exp on ACT in [128, 2, TT] batches (the hard floor: the
    scalar engine is the only exp engine, NH*S*L/128 columns ~ 55us).
    Denominators ride per-head indicator matmuls accumulated per quad
    into psum rows {0,1,32,33}, one f32r reciprocal per quad, partition
    broadcast by a tiny fp32r matmul per pair, materialized to SBUF on
    ACT, then a single DVE multiply normalizes each av pair.
  - Flat cross-tile software pipeline: tile t's attention overlaps tile
    t-1's out-projection/residual and tile t+1's Q projection; PSUM is
    exactly 8 banks (dots ring 2x[128,2,TT], av ring 2, a shared bank
    for the denominator/broadcast psums, one for psq/pso).
"""

import sys

import numpy as np

sys.path.insert(0, "/opt/trn_rl_repo")

import ml_dtypes

import concourse.bacc as bacc
import concourse.bass as bass
import concourse.mybir as mybir
import concourse.tile as tile
from concourse.bass_utils import run_bass_kernel_spmd

F32 = mybir.dt.float32
F32R = mybir.dt.float32r
BF16 = mybir.dt.bfloat16
F8 = mybir.dt.float8e4
AF = mybir.ActivationFunctionType
OP = mybir.AluOpType
DR = mybir.MatmulPerfMode.DoubleRow

B = 8
C = 512
L = 4096          # 64*64
CC = 768
S = 256
INNER = 512
NH = 8
DH = 64
G = 32
EPS = 1e-5
TT = 512          # t-tile
NT = L // TT      # 8
NCORES = 8
SCALE2 = 1.0 / DH

NXB = 4           # x channel blocks (kc2, j)
NCB = 6           # ctx channel blocks
NQB = 4           # q/k out blocks (mq2, j)

# packed per-partition vector columns
VOFF = {"bq": 0, "bkvk": 4, "bo": 8, "gxg": 12, "gxb": 16, "gcg": 20,
        "gcb": 26, "eps": 32}
VCOLS = 36


def _r(ap):
    return ap.bitcast(F32R)


def _emit(nc, tc, d):
    sync = nc.sync
    act = nc.scalar
    dve = nc.vector
    pe = nc.tensor
    gp = nc.gpsimd
    ds = bass.ds

    with tc.tile_pool(name="keep", bufs=1) as keep:
        # ---------------- persistent tiles ----------------
        xall = keep.tile([128, 2, 2, L], BF16, name="xall", tag="xall")
        x8 = keep.tile([128, 2, 2, L], F8, name="x8", tag="x8")
        wq8 = keep.tile([128, 2, 2, INNER], F8, name="wq8", tag="wq8")
        wo8 = keep.tile([128, 2, 2, C], F8, name="wo8", tag="wo8")
        k8 = keep.tile([128, 4, S], F8, name="k8", tag="k8")
        v8 = keep.tile([128, 2, NH, 128], F8, name="v8", tag="v8")
        ones8 = keep.tile([128, 2, NH, 36], F8, name="ones8", tag="ones8")
        patP = keep.tile([36, 2, 128], F32, name="patP", tag="patP")
        vecs = keep.tile([128, VCOLS], F32, name="vecs", tag="vecs")
        bqe = keep.tile([128, NQB], F32, name="bqe", tag="bqe")
        rowm = keep.tile([1, 128 + INNER], F32, name="rowm", tag="rowm")

        def vcol(nm, j=0):
            return vecs[:, VOFF[nm] + j:VOFF[nm] + j + 1]

        with tc.tile_pool(name="sb0", bufs=1) as sb0, \
             tc.tile_pool(name="ps0", bufs=1, space="PSUM") as ps0:

            ctx_sb = sb0.tile([128, 3, 2, S], BF16, name="ctx_sb", tag="ctx_sb")
            gnc8 = sb0.tile([128, 3, 2, S], F8, name="gnc8", tag="gnc8")
            indall = sb0.tile([128, NCB + NXB, G], F32, name="indall",
                              tag="indall")
            indTall = sb0.tile([G, (NCB + NXB) * 128], F32, name="indTall",
                               tag="indTall")
            wqT_sb = sb0.tile([128, 2, 2, INNER], F32, name="wqT_sb",
                              tag="wqT_sb")
            wkv8 = sb0.tile([128, 3, 2, 2 * INNER], F8, name="wkv8", tag="wkv8")

            # ---- DMA schedule: x first (the stats chain gates phase A) ----
            xv = d["x"].rearrange("p (a b l) -> p a b l", a=2, b=2)
            for kc2 in range(2):
                for j in range(2):
                    if kc2 == 0:
                        sync.dma_start(xall[:, kc2, j, :], xv[:, kc2, j, :])
                    else:
                        for hf in range(2):
                            hsl = ds(hf * (L // 2), L // 2)
                            sync.dma_start(xall[:, kc2, j, hsl],
                                           xv[:, kc2, j, hsl])
            sync.dma_start(ctx_sb[:, :, :, :], d["ctx"].rearrange(
                "p (a b s) -> p a b s", a=3, b=2))
            sync.dma_start(_r(indall[:, :, :]),
                           _r(d["indall"].rearrange("(j p) g -> p j g", p=128)))
            sync.dma_start(_r(indTall[:, :]), _r(d["indTall"][:, :]))
            sync.dma_start(vecs[:, :], d["vecs"][:, :])
            sync.dma_start(_r(rowm[:, :]), _r(d["rowm"][:, :]))
            sync.dma_start(_r(wqT_sb[:, :, :, :]), _r(d["wqT"].rearrange(
                "p (a b o) -> p a b o", a=2, b=2)))
            sync.dma_start(wkv8[:, :, :, :], d["wkv8"].rearrange(
                "p (a b o) -> p a b o", a=3, b=2))
            sync.dma_start(wo8[:, :, :, :], d["wo8"].rearrange(
                "p (a b o) -> p a b o", a=2, b=2))
            sync.dma_start(ones8[:, :, :, :], d["ones8"].rearrange(
                "p (a h r) -> p a h r", a=2, h=NH))
            sync.dma_start(_r(patP[:, :, :]), _r(d["patP"].rearrange(
                "r (q c) -> r q c", q=2)))

            gp.memset(v8[:, :, :, :], 0.0)

            ind_c = [indall[:, j, :] for j in range(NCB)]
            ind_x = [indall[:, NCB + m, :] for m in range(NXB)]

            def indT_c(j):
                return indTall[:, j * 128:(j + 1) * 128]

            def indT_x(m):
                return indTall[:, (NCB + m) * 128:(NCB + m + 1) * 128]

            ones_row = rowm[0:1, 0:128]
            bkvv_row = rowm[0:1, 128:128 + INNER]

            def chan_stats(src, nblks, nsub, sub, ind_tiles, inv_n, tagp,
                           pre_rhs=()):
                bns = []
                for jb in range(nblks):
                    bn = sb0.tile([128, nsub * 6], F32, name=f"bn{tagp}{jb}",
                                  tag=f"bn{tagp}{jb}")
                    bns.append(bn.rearrange("p (a q) -> p a q", q=6))
                for jb in range(nblks):
                    for a in range(nsub):
                        dve.bn_stats(bns[jb][:, a, :],
                                     src(jb)[:, a * sub:(a + 1) * sub])
                rhs_list = []
                for jb in range(nblks):
                    st = sb0.tile([128, 2], F32, name=f"st{tagp}{jb}",
                                  tag=f"st{tagp}{jb}")
                    dve.bn_aggr(st[:, :], bns[jb])
                    r2 = sb0.tile([128, 2], F32, name=f"r2{tagp}{jb}",
                                  tag=f"r2{tagp}{jb}")
                    dve.tensor_copy(_r(r2[:, 0:1]), st[:, 0:1])
                    dve.scalar_tensor_tensor(_r(r2[:, 1:2]), st[:, 0:1],
                                             st[:, 0:1], st[:, 1:2],
                                             op0=OP.mult, op1=OP.add)
                    rhs_list.append(r2)
                psg = ps0.tile([G, 2], F32, name=f"psg{tagp}", tag="misc",
                               bufs=2)
                pairs = [(ind, mk()) for ind, mk in pre_rhs]
                pairs += [(ind_tiles[jb], rhs_list[jb]) for jb in range(nblks)]
                for i, (ind, r2) in enumerate(pairs):
                    pe.matmul(psg[:, :], ind, r2[:, :],
                              start=(i == 0), stop=(i == len(pairs) - 1))
                gstat = sb0.tile([G, 2], F32, name=f"gstat{tagp}",
                                 tag=f"gstat{tagp}")
                act.mul(gstat[:, :], psg[:, :], inv_n)
                nvar = sb0.tile([G, 1], F32, name=f"nvar{tagp}",
                                tag=f"nvar{tagp}")
                dve.scalar_tensor_tensor(nvar[:, :], gstat[:, 0:1],
                                         gstat[:, 0:1], gstat[:, 1:2],
                                         op0=OP.mult, op1=OP.subtract)
                sd = sb0.tile([G, 1], F32, name=f"sd{tagp}", tag=f"sd{tagp}")
                act.activation(sd[:, :], nvar[:, :], AF.Sqrt, scale=-1.0,
                               bias=vcol("eps")[0:G, :])
                er = sb0.tile([G, 2], F32, name=f"er{tagp}", tag=f"er{tagp}")
                with nc.allow_low_precision(reason="f32r bitcast is 32-bit"):
                    dve.reciprocal(_r(er[:, 0:1]), sd[:, :])
                dve.tensor_copy(_r(er[:, 1:2]), gstat[:, 0:1])
                return er

            def expand_ab(er, indT, jb, gam, bet, tagp):
                pse = ps0.tile([128, 2], F32, name=f"pse{tagp}{jb}", tag="misc",
                               bufs=2)
                pe.matmul(pse[:, :], indT, er[:, :], start=True, stop=True)
                A = sb0.tile([128, 1], F32, name=f"A{tagp}{jb}",
                             tag=f"A{tagp}{jb}")
                dve.tensor_tensor(A[:, :], pse[:, 0:1], gam, op=OP.mult)
                Bt = sb0.tile([128, 1], F32, name=f"B{tagp}{jb}",
                              tag=f"B{tagp}{jb}")
                muA = sb0.tile([128, 1], F32, name=f"muA{tagp}{jb}",
                               tag=f"muA{tagp}", bufs=2)
                dve.tensor_tensor(muA[:, :], pse[:, 1:2], A[:, :], op=OP.mult)
                dve.tensor_tensor(_r(Bt[:, :]), bet, muA[:, :], op=OP.subtract)
                return A, Bt

            # ---- context groupnorm (materialized, fp8 out) ----
            er_c = chan_stats(lambda jb: ctx_sb[:, jb // 2, jb % 2, :], NCB, 1,
                              S, ind_c, 1.0 / (CC // G), "c")
            for jb in range(NCB):
                A, Bt = expand_ab(er_c, indT_c(jb), jb, vcol("gcg", jb),
                                  vcol("gcb", jb), "c")
                dve.tensor_scalar(gnc8[:, jb // 2, jb % 2, :],
                                  ctx_sb[:, jb // 2, jb % 2, :], A[:, :],
                                  Bt[:, :], op0=OP.mult, op1=OP.add)

            # ---- x8 quantize: blocks 0-1 are produced whole by the ACT
            # stats pass below; gpsimd casts blocks 2-3 tile-by-tile ----
            x8v = x8.rearrange("p a b l -> p (a b) l")
            xallv = xall.rearrange("p a b l -> p (a b) l")

            def x8_cast(ti):
                tsl = ds(ti * TT, TT)
                gp.tensor_copy(x8v[:, 2:4, tsl], xallv[:, 2:4, tsl])

            # ACT: fused cast+sum and square+sum for x blocks 0 and 1
            accx = sb0.tile([128, 2, 2], F32, name="accx", tag="accx")
            scrx = sb0.tile([128, L], BF16, name="scrx", tag="scrx")
            for jb in range(2):
                act.activation(x8v[:, jb, :], xallv[:, jb, :], AF.Copy,
                               accum_out=accx[:, jb, 0:1])
                act.activation(scrx[:, :], xallv[:, jb, :], AF.Square,
                               accum_out=accx[:, jb, 1:2])

            for ti in range(2):
                x8_cast(ti)

            # ---- x stats -> fold into wq8 / bqe ----
            # blocks 0,1 come from the ACT accumulators; 2,3 from bn_stats
            def xr2_act(jb):
                r2 = sb0.tile([128, 2], F32, name=f"r2xa{jb}", tag=f"r2xa{jb}")
                dve.tensor_scalar(r2[:, :], accx[:, jb, :], 1.0 / L, None,
                                  op0=OP.mult)
                return r2
            er_x = chan_stats(lambda jb: xall[:, 1, jb, :], 2, 8,
                              512, ind_x[2:], 1.0 / (C // G), "x",
                              pre_rhs=[(ind_x[0], lambda: xr2_act(0)),
                                       (ind_x[1], lambda: xr2_act(1))])
            Bx = []
            for jb in range(NXB):
                A, Bt = expand_ab(er_x, indT_x(jb), jb, vcol("gxg", jb),
                                  vcol("gxb", jb), "x")
                act.activation(wq8[:, jb // 2, jb % 2, :],
                               wqT_sb[:, jb // 2, jb % 2, :], AF.Copy,
                               scale=A[:, :])
                Bx.append(Bt)
            for b in range(NQB):
                psb = ps0.tile([128, 1], F32, name=f"psb{b}", tag="misc",
                               bufs=2)
                for jb in range(NXB):
                    pe.matmul(psb[:, :],
                              wqT_sb[:, jb // 2, jb % 2,
                                     b * 128:(b + 1) * 128],
                              Bx[jb][:, :], start=(jb == 0),
                              stop=(jb == NXB - 1))
                dve.tensor_tensor(bqe[:, b:b + 1], psb[:, :], vcol("bq", b),
                                  op=OP.add)

            # ---- K and V^T projections (fp8 DoubleRow) ----
            psk = [ps0.tile([128, S], F32, name=f"psk{b}", tag=f"psk{b}")
                   for b in range(NQB)]
            psv = [ps0.tile([128, INNER], F32, name=f"psv{sc}", tag=f"psv{sc}")
                   for sc in range(2)]
            for kc2 in range(3):
                for b in range(NQB):
                    pe.matmul(psk[b][:, :],
                              wkv8[:, kc2, :, b * 128:(b + 1) * 128],
                              gnc8[:, kc2, :, :], start=(kc2 == 0),
                              stop=(kc2 == 2), perf_mode=DR)
                for sc in range(2):
                    pe.matmul(psv[sc][:, :],
                              gnc8[:, kc2, :, sc * 128:(sc + 1) * 128],
                              wkv8[:, kc2, :, INNER:2 * INNER],
                              start=(kc2 == 0), stop=False, perf_mode=DR)
            for sc in range(2):
                pe.matmul(psv[sc][:, :], _r(ones_row), _r(bkvv_row),
                          start=False, stop=True)
            for b in range(NQB):
                act.activation(k8[:, b, :], psk[b][:, :],
                               AF.Identity, bias=vcol("bkvk", b))
            for sc in range(2):
                for par in range(2):
                    dve.tensor_copy(v8[:, sc, par::2, par * DH:(par + 1) * DH],
                                    psv[sc].rearrange("p (h c) -> p h c",
                                                      c=DH)[:, par::2, :])

        # ---------------- t-loop: Q / attention / out-proj ----------------
        with tc.tile_pool(name="work", bufs=1) as work, \
             tc.tile_pool(name="ps1", bufs=1, space="PSUM") as ps1:
            out_view = d["out"].rearrange("(a p b) l -> p a b l", p=128, b=2)
            # Flat cross-tile software pipeline. PSUM (8 banks): psd ring
            # 2x[128,2,TT] (4) + big ring 3x[128,TT] (3, shared by av, bc,
            # psq and pso) + psg32 (1). Heads processed as two quads; the
            # denominator gather lands at psg32 offsets 0/64 (the only legal
            # matmul output base partitions besides 32).
            q8_t = {}
            E_t = {}
            psg_t = {}
            av_t = {}
            avn_t = {}

            def emit_q(t, b):
                tsl = ds(t * TT, TT)
                if b == 0:
                    q8_t[t] = work.tile([128, 4, TT], F8, name=f"q8_{t}",
                                        tag="q8", bufs=3)
                if t == 0:
                    psq = ps1.tile([128, TT], F32, name=f"psq{t}_{b}",
                                   tag="av", bufs=2)
                else:
                    psq = ps1.tile([128, TT], F32, name=f"psq{t}_{b}",
                                   tag="qo", bufs=1)
                for kc2 in range(2):
                    pe.matmul(psq[:, :], wq8[:, kc2, :, b * 128:(b + 1) * 128],
                              x8[:, kc2, :, tsl], start=(kc2 == 0),
                              stop=(kc2 == 1), perf_mode=DR)
                dve.tensor_scalar_add(q8_t[t][:, b, :], psq[:, :],
                                      bqe[:, b:b + 1])

            def dots_exp(t, h):
                psd = ps1.tile([128, 2, TT], F32, name=f"psd{t}_{h}",
                               tag="psd", bufs=2)
                h4 = (h % 2) * DH
                q8 = q8_t[t]
                for sc in range(2):
                    pe.matmul(psd[:, sc, :],
                              k8[h4:h4 + DH, h // 2,
                                 sc * 128:(sc + 1) * 128],
                              q8[h4:h4 + DH, h // 2, :],
                              start=True, stop=True)
                E8 = work.tile([128, 2, TT], F8, name=f"E{t}_{h}", tag="E",
                               bufs=4)
                act.activation(E8[:, :, :], psd[:, :, :], AF.Exp,
                               scale=SCALE2)
                E_t[(t, h)] = E8

            def av_mm(t, p2):
                """AV + denominator-gather matmuls for head pair p2. Each
                quad's denominators accumulate into one [36, TT] psum at
                rows {0, 1, 32, 33}; the quad's two av pairs land in one
                [128, 2, TT] psum tile."""
                if p2 == 0:
                    avn_t[t] = work.tile([128, 4, TT], F8, name=f"avn{t}",
                                         tag="avn", bufs=3)
                if p2 % 2 == 0:
                    psg_t[t] = ps1.tile([36, TT], F32,
                                        name=f"psg{t}_{p2 // 2}",
                                        tag="m1", bufs=1)
                psgQ = psg_t[t]
                av = ps1.tile([128, TT], F32, name=f"av{t}_{p2}", tag="av",
                              bufs=2)
                for r2 in range(2):
                    h = 2 * p2 + r2
                    E8 = E_t.pop((t, h))
                    pe.matmul(av[:, :], v8[:, :, h, :], E8[:, :, :],
                              start=(r2 == 0), stop=(r2 == 1), perf_mode=DR)
                    pe.matmul(psgQ[:, :], ones8[:, :, h, :], E8[:, :, :],
                              start=(h % 4 == 0), stop=(h % 4 == 3),
                              perf_mode=DR)
                return av

            def rcp_quad(t, qd):
                rcpQ = work.tile([36, TT], F32, name=f"rcq{t}_{qd}", tag="rcp",
                                 bufs=2)
                with nc.allow_low_precision(reason="f32r bitcast is 32-bit"):
                    dve.reciprocal(_r(rcpQ[:, :]), psg_t[t][:, :])
                return rcpQ

            def norm_pair(t, p2, av, rcpQ):
                """broadcast 1/denom for pair p2 (PE) and normalize its av."""
                bc = ps1.tile([128, TT], F32, name=f"bc{t}_{p2}", tag="m1",
                              bufs=1)
                pe.matmul(bc[:, :], _r(patP[:, p2 % 2, :]), _r(rcpQ[:, :]),
                          start=True, stop=True)
                bcs = work.tile([128, TT], F32, name=f"bcs{t}_{p2}", tag="bcs",
                                bufs=4)
                act.copy(bcs[:, :], bc[:, :])
                dve.tensor_tensor(avn_t[t][:, p2, :], av[:, :], bcs[:, :],
                                  op=OP.mult)

            out_t = {}

            def emit_o(t, m):
                tsl = ds(t * TT, TT)
                if m == 0:
                    out_t[t] = work.tile([128, 2, 2, TT], F32, name=f"o{t}",
                                         tag="osb", bufs=3)
                if t == NT - 1:
                    pso = ps1.tile([128, 2, TT], F32, name=f"pso{t}_{m}",
                                   tag="psd", bufs=2)[:, 0, :]
                else:
                    pso = ps1.tile([128, TT], F32, name=f"pso{t}_{m}",
                                   tag="qo", bufs=1)
                for kq2 in range(2):
                    pe.matmul(pso[:, :], wo8[:, kq2, :, m * 128:(m + 1) * 128],
                              avn_t[t][:, 2 * kq2:2 * kq2 + 2, :],
                              start=(kq2 == 0), stop=(kq2 == 1), perf_mode=DR)
                eng = dve
                eng.scalar_tensor_tensor(out_t[t][:, m // 2, m % 2, :],
                                         pso[:, :], vcol("bo", m),
                                         xall[:, m // 2, m % 2, tsl],
                                         op0=OP.add, op1=OP.add)
                sync.dma_start(out_view[:, m // 2, m % 2, tsl],
                               out_t[t][:, m // 2, m % 2, :])
                if m == 3:
                    out_t.pop(t)
                    avn_t.pop(t)

            # prologue
            for b in range(NQB):
                emit_q(0, b)
            dots_exp(0, 0)
            dots_exp(0, 1)
            for t in range(NT):
                for qd in range(2):
                    pa, pb = 2 * qd, 2 * qd + 1
                    dots_exp(t, 4 * qd + 2)
                    dots_exp(t, 4 * qd + 3)
                    av_a = av_mm(t, pa)
                    if qd == 0:
                        dots_exp(t, 4)
                        dots_exp(t, 5)
                    elif t + 1 < NT:
                        emit_q(t + 1, 0)
                        emit_q(t + 1, 1)
                        dots_exp(t + 1, 0)
                        dots_exp(t + 1, 1)
                    av_b = av_mm(t, pb)
                    rcpQ = rcp_quad(t, qd)
                    norm_pair(t, pa, av_a, rcpQ)
                    norm_pair(t, pb, av_b, rcpQ)
                    if t > 0:
                        emit_o(t - 1, 2 * qd)
                        emit_o(t - 1, 2 * qd + 1)
                if t + 1 < NT:
                    if t + 2 < NT:
                        x8_cast(t + 2)
                    emit_q(t + 1, 2)
                    emit_q(t + 1, 3)
            for m in range(4):
                emit_o(NT - 1, m)


_CACHE = {}


def _build():
    if "nc" in _CACHE:
        return _CACHE["nc"]
    nc = bacc.Bacc("TRN2", target_bir_lowering=False, debug=False,
                   num_devices=NCORES)
    d = {}
    d["x"] = nc.dram_tensor("x", [128, 4 * L], BF16, kind="ExternalInput").ap()
    d["ctx"] = nc.dram_tensor("ctx", [128, 6 * S], BF16,
                              kind="ExternalInput").ap()
    d["wqT"] = nc.dram_tensor("wqT", [128, 4 * INNER], F32,
                              kind="ExternalInput").ap()
    d["wkv8"] = nc.dram_tensor("wkv8", [128, 6 * INNER * 2], F8,
                               kind="ExternalInput").ap()
    d["wo8"] = nc.dram_tensor("wo8", [128, 4 * C], F8,
                              kind="ExternalInput").ap()
    d["vecs"] = nc.dram_tensor("vecs", [128, VCOLS], F32,
                               kind="ExternalInput").ap()
    d["indall"] = nc.dram_tensor("indall", [(NCB + NXB) * 128, G], F32,
                                 kind="ExternalInput").ap()
    d["indTall"] = nc.dram_tensor("indTall", [G, (NCB + NXB) * 128], F32,
                                  kind="ExternalInput").ap()
    d["ones8"] = nc.dram_tensor("ones8", [128, 2 * NH * 36], F8,
                                kind="ExternalInput").ap()
    d["patP"] = nc.dram_tensor("patP", [36, 2 * 128], F32,
                               kind="ExternalInput").ap()
    d["rowm"] = nc.dram_tensor("rowm", [1, 128 + INNER], F32,
                               kind="ExternalInput").ap()
    d["out"] = nc.dram_tensor("out", [C, L], F32, kind="ExternalOutput").ap()

    with tile.TileContext(nc) as tc:
        _emit(nc, tc, d)
    nc.compile()
    _CACHE["nc"] = nc
    return nc


# ---- host-side orderings ----
def _x_chan(kc2, p, j):
    return kc2 * 256 + 2 * p + j


def _q_chan(b, c):
    mq2, jq = b // 2, b % 2
    return (mq2 * 4 + c // 32) * 64 + (c % 32) * 2 + jq


def _host_inputs(inputs):
    f = np.float32
    bf = ml_dtypes.bfloat16
    f8 = ml_dtypes.float8_e4m3fn

    x = np.asarray(inputs["x"], dtype=f).reshape(B, C, L)
    ctx = np.asarray(inputs["context"], dtype=f)
    wq = np.asarray(inputs["wq"], dtype=f)
    wkv = np.asarray(inputs["wkv"], dtype=f)
    wo = np.asarray(inputs["wo"], dtype=f)
    bkv = np.asarray(inputs["bkv"], dtype=f)

    p_ = np.arange(128)
    # x/out channel order: channel(p; kc2, j) = kc2*256 + 2p + j
    xch = np.empty((2, 128, 2), np.int64)
    for kc2 in range(2):
        for j in range(2):
            xch[kc2, :, j] = _x_chan(kc2, p_, j)
    xperm = xch.transpose(1, 0, 2).reshape(128, 4)     # [p, (kc2,j)]
    # ctx channel order
    cch = np.empty((3, 128, 2), np.int64)
    for kc2 in range(3):
        for j in range(2):
            cch[kc2, :, j] = kc2 * 256 + 2 * p_ + j
    cperm = cch.transpose(1, 0, 2).reshape(128, 6)     # [p, (kc2,j)]
    # q/k inner order: natural (head h at rows (h%2)*64 of block h//2)
    qcols = np.arange(NQB * 128).reshape(NQB, 128)
    qorder = qcols.reshape(-1)

    # x_dev [128, (kc2,j,L)] bf16
    x_dev = np.empty((B, 128, 2, 2, L), bf)
    for kc2 in range(2):
        for j in range(2):
            x_dev[:, :, kc2, j, :] = x[:, xch[kc2, :, j], :].astype(bf)
    x_dev = x_dev.reshape(B, 128, 4 * L)

    # ctx_dev [128, (kc2,j,S)] bf16
    ctx_dev = np.empty((B, 128, 3, 2, S), bf)
    for kc2 in range(3):
        for j in range(2):
            ctx_dev[:, :, kc2, j, :] = ctx[:, cch[kc2, :, j], :].astype(bf)
    ctx_dev = ctx_dev.reshape(B, 128, 6 * S)

    # wqT_dev [128, (kc2,j,512cols)] f32 : wq[qorder(col), xchan(p,kc2,j)]
    wqT_dev = np.empty((128, 2, 2, INNER), f)
    for kc2 in range(2):
        for j in range(2):
            wqT_dev[:, kc2, j, :] = wq[np.ix_(qorder, xch[kc2, :, j])].T
    wqT_dev = wqT_dev.reshape(128, 4 * INNER)

    # wkv8_dev [128, (kc2,j, k512 | v512)] fp8
    wkv8_dev = np.empty((128, 3, 2, 2 * INNER), f)
    vorder = np.arange(INNER) + INNER        # natural v rows of wkv
    for kc2 in range(3):
        for j in range(2):
            cc = cch[kc2, :, j]
            wkv8_dev[:, kc2, j, :INNER] = wkv[np.ix_(qorder, cc)].T
            wkv8_dev[:, kc2, j, INNER:] = wkv[np.ix_(vorder, cc)].T
    wkv8_dev = wkv8_dev.reshape(128, 12 * INNER).astype(f8)

    # wo8_dev [128, (kq2,jq, 512cols)] fp8 : wo[outchan(col), (2kq2+jq)*128+p]
    outcols = np.empty((4, 128), np.int64)
    for bo in range(4):
        outcols[bo] = _x_chan(bo // 2, np.arange(128), bo % 2)
    wo8_dev = np.empty((128, 2, 2, C), f)
    for kq2 in range(2):
        for jq in range(2):
            inner_idx = (2 * kq2 + jq) * 128 + p_
            wo8_dev[:, kq2, jq, :] = wo[np.ix_(outcols.reshape(-1),
                                               inner_idx)].T
    wo8_dev = wo8_dev.reshape(128, 4 * C).astype(f8)

    # indicator matrices (permuted orders)
    ind_x = np.zeros((NXB, 128, G), f)
    indT_x = np.zeros((NXB, G, 128), f)
    for blk in range(NXB):
        g = xch[blk // 2, :, blk % 2] // (C // G)
        ind_x[blk, p_, g] = 1.0
        indT_x[blk, g, p_] = 1.0
    ind_c = np.zeros((NCB, 128, G), f)
    indT_c = np.zeros((NCB, G, 128), f)
    for blk in range(NCB):
        g = cch[blk // 2, :, blk % 2] // (CC // G)
        ind_c[blk, p_, g] = 1.0
        indT_c[blk, g, p_] = 1.0
    indall = np.concatenate([ind_c, ind_x], axis=0).reshape(-1, G)
    indall = np.ascontiguousarray(indall)
    indTall = np.ascontiguousarray(
        np.concatenate([indT_c, indT_x], axis=0).transpose(1, 0, 2)
        .reshape(G, -1))

    # gather patterns: quad pairs land at rows {0,1} and {32,33}.
    # Unused rows also get a (real) denominator so 1/x never sees a zero.
    ones8 = np.zeros((128, 2, NH, 36), f)
    for h in range(NH):
        ones8[:, :, h, 32 * ((h % 4) // 2) + h % 2] = 1.0
        if h % 4 == 0:
            ones8[:, :, h, 2:32] = 1.0
        if h % 4 == 2:
            ones8[:, :, h, 34:36] = 1.0
    ones8 = ones8.reshape(128, 2 * NH * 36).astype(f8)

    # patP[r, pq, c] = (r == 32*pq + (c >= 64))
    patP = np.zeros((36, 2, 128), f)
    for pq in range(2):
        patP[32 * pq, pq, 0:DH] = 1.0
        patP[32 * pq + 1, pq, DH:128] = 1.0
    patP = patP.reshape(36, 2 * 128)


    def cols_perm(vec, order):
        return np.asarray(vec, dtype=f)[order]          # [128, n]

    vecs = np.zeros((128, VCOLS), f)
    vecs[:, VOFF["bq"]:VOFF["bq"] + 4] = cols_perm(inputs["bq"], qcols.T)
    vecs[:, VOFF["bkvk"]:VOFF["bkvk"] + 4] = cols_perm(bkv[:INNER], qcols.T)
    vecs[:, VOFF["bo"]:VOFF["bo"] + 4] = cols_perm(inputs["bo"],
                                                   outcols.T)
    vecs[:, VOFF["gxg"]:VOFF["gxg"] + 4] = cols_perm(inputs["gnx_g"], xperm)
    vecs[:, VOFF["gxb"]:VOFF["gxb"] + 4] = cols_perm(inputs["gnx_b"], xperm)
    vecs[:, VOFF["gcg"]:VOFF["gcg"] + 6] = cols_perm(inputs["gnc_g"], cperm)
    vecs[:, VOFF["gcb"]:VOFF["gcb"] + 6] = cols_perm(inputs["gnc_b"], cperm)
    vecs[:, VOFF["eps"]] = EPS

    rowm = np.zeros((1, 128 + INNER), f)
    rowm[0, :128] = 1.0
    rowm[0, 128:] = bkv[INNER:]

    shared = {
        "wqT": wqT_dev,
        "wkv8": wkv8_dev,
        "wo8": wo8_dev,
        "indall": indall,
        "indTall": indTall,
        "ones8": ones8,
        "patP": patP,
        "rowm": rowm,
        "vecs": vecs,
    }
    return [{"x": x_dev[i], "ctx": ctx_dev[i], **shared} for i in range(NCORES)]


def run(inputs, **spmd_kwargs):
    nc = _build()
    in_maps = _host_inputs(inputs)
    res = run_bass_kernel_spmd(nc, in_maps, list(range(NCORES)), **spmd_kwargs)
    out = np.stack([np.asarray(res.results[i]["out"], dtype=np.float32)
                    for i in range(NCORES)])
    return out.reshape(B, C, 64, 64), res


def kernel(**inputs) -> np.ndarray:
    out, _ = run(inputs)
    return out
